# revision 5
# baseline (speedup 1.0000x reference)
"""BlockwiseDense Trainium2 kernel (8 NeuronCores, sharded over out_blocks).

Math (per reference):
    w = rram_quantize(relu(cores))          # snap to 256 log-spaced levels
    y[b,i,j,k] = sum_l w[i,j,k,l] * x[b,j,l]

The quantizer index s(w) = MULT*ln((A-w)/B) + C0 is approximated by the
same quadratic the baseline used, but factored into linear terms
    s ~= (A2M*w + P1F) * (w + CF)          [~0.06% level flips]
so the n-stage is TWO fused ops instead of three:
    gg = ts(w, A2M, P1F)            (DVE, fp16, 2x rate)
    n  = sat_u8(stt(w + CF) * gg)   (GpSimd scalar_tensor_tensor)
then e = Exp(n*ln_r) fp16 (ACT, the only ACT work); fp16 matmuls
accumulate in fp32 PSUM over the two 128-row halves of l; per-j evict
applies y = -B*(x@e) + A*s2 on DVE (s2 = row-sums of x via a ones
matmul, scaled on GpSimd).

relu is implicit: negative w gives s<0 which the u8 saturating cast
clamps to n=0 (the g_min level), exactly matching relu+quantize.

DMA: all 10 weight-granule loads are issued upfront on the sync ring so
the 16 HW DMA engines stay descriptor-fed (~350 GB/s aggregate); x is
split into two chunks on the gpsimd ring; y stores ride the sync ring
behind the weights.  Core c takes out_blocks {2c, 2c+1}.
"""

import numpy as np

import concourse.bacc as bacc
import concourse.mybir as mybir
from concourse.tile import TileContext
from concourse.bass_utils import run_bass_kernel_spmd

BATCH = 128
IN_BLOCKS = 16
OUT_BLOCKS = 16
NB = 256
N_CORES = 8
I_PER_CORE = OUT_BLOCKS // N_CORES  # 2
IK = I_PER_CORE * NB  # 512

TAU, G_INF, G_MIN, L = 0.75, 2.0, 0.001, 256
B_SCALE = (G_INF - G_MIN) / (1.0 - float(np.exp(-TAU)))
A_OFF = G_MIN + B_SCALE
MULT = -(L - 1) / TAU
LN_R = -TAU / (L - 1)

# quadratic s(w) ~= A2M*w^2 + P1*w + P0 (baseline fit), factored as
# s = (A2M*w + P1F)*(w + CF) with CF the small root of A2M c^2 - P1 c + P0
C0 = 0.5 - float(np.log((1 + np.exp(LN_R)) / 2) / LN_R)
_c1 = -C0 / 340.0
_g2 = -340.0 - 170.0 * _c1
C1W = -(G_MIN + B_SCALE * _c1)
A2M = 170.0 / (B_SCALE * B_SCALE)
A2B = -(170.0 * G_MIN / B_SCALE + _g2) / B_SCALE
P1 = A2B + C1W * A2M
P0 = C1W * A2B
CF = (P1 - float(np.sqrt(P1 * P1 - 4.0 * A2M * P0))) / (2.0 * A2M)
P1F = P1 - A2M * CF

F32 = mybir.dt.float32
F16 = mybir.dt.float16
U8 = mybir.dt.uint8

# granules: contiguous ascending j-lists; small head/tail for pipeline
GSPEC = [[0], [1], [2, 3], [4, 5], [6, 7], [8, 9], [10, 11], [12, 13], [14], [15]]

_CACHE = {}


def _build():
    nc = bacc.Bacc(trn_type="TRN2")
    P = 128
    NG = len(GSPEC)

    xt_d = nc.dram_tensor("xt", [P, IN_BLOCKS, 2, BATCH], F16, kind="ExternalInput")
    wt_d = nc.dram_tensor("wt", [P, IN_BLOCKS, 2, IK], F16, kind="ExternalInput")
    y_d = nc.dram_tensor("y", [BATCH, IN_BLOCKS, IK], F16, kind="ExternalOutput")

    flat = "p a b k -> p (a b k)"

    with TileContext(nc) as tc:
        with (
            tc.tile_pool(name="singles", bufs=1) as singles,
            tc.tile_pool(name="wraw", bufs=NG) as wpool,
            tc.tile_pool(name="tgg", bufs=4) as tpool,
            tc.tile_pool(name="nidx", bufs=4) as npool,
            tc.tile_pool(name="texp", bufs=4) as epool,
            tc.tile_pool(name="sa", bufs=4) as spool,
            tc.tile_pool(name="yout", bufs=3) as ypool,
            tc.tile_pool(name="yps", bufs=6, space="PSUM") as yps,
            tc.tile_pool(name="sps", bufs=1, space="PSUM") as sps,
        ):
            wt_t = [None] * NG
            g_t = [None] * NG
            n_t = [None] * NG
            e_t = [None] * NG
            sa_t = [None] * NG
            y_t = [None] * NG
            p_t = [None] * IN_BLOCKS

            def dma_w(g):
                js = GSPEC[g]
                nj = len(js)
                wt_t[g] = wpool.tile([P, nj, 2, IK], F16, name="wraw", tag="wraw")
                nc.sync.dma_start(out=wt_t[g][:], in_=wt_d[:, js[0] : js[0] + nj])

            def gg_stage(g):
                js = GSPEC[g]
                fd = len(js) * 2 * IK
                g_t[g] = tpool.tile([P, fd], F16, name="tgg", tag="tgg")
                nc.gpsimd.tensor_scalar(
                    g_t[g][:],
                    wt_t[g][:].rearrange(flat),
                    A2M,
                    P1F,
                    mybir.AluOpType.mult,
                    mybir.AluOpType.add,
                )

            def n_stage(g):
                js = GSPEC[g]
                fd = len(js) * 2 * IK
                n_t[g] = npool.tile([P, fd], U8, name="nidx", tag="nidx")
                nc.vector.scalar_tensor_tensor(
                    n_t[g][:],
                    wt_t[g][:].rearrange(flat),
                    CF,
                    g_t[g][:],
                    mybir.AluOpType.add,
                    mybir.AluOpType.mult,
                )

            def exp_stage(g):
                js = GSPEC[g]
                nj = len(js)
                e_t[g] = epool.tile([P, nj, 2, IK], F16, name="texp", tag="texp")
                nc.scalar.activation(
                    e_t[g][:].rearrange(flat),
                    n_t[g][:],
                    mybir.ActivationFunctionType.Exp,
                    bias=0.0,
                    scale=LN_R,
                )

            def mm_stage(g):
                js = GSPEC[g]
                for jrel, j in enumerate(js):
                    p_t[j] = yps.tile([P, IK], F32, name="yp", tag="yp")
                    for h in range(2):
                        nc.tensor.matmul(
                            s2_ps[:, j : j + 1],
                            xt_sb[:, j, h, :],
                            ones_sb[:],
                            start=(h == 0),
                            stop=(h == 1),
                        )
                        nc.tensor.matmul(
                            p_t[j][:],
                            xt_sb[:, j, h, :],
                            e_t[g][:, jrel, h, :],
                            start=(h == 0),
                            stop=(h == 1),
                        )

            def sa_stage(g):
                js = GSPEC[g]
                nj = len(js)
                sa_t[g] = spool.tile([P, nj], F32, name="sa", tag="sa")
                nc.vector.tensor_scalar(
                    sa_t[g][:],
                    s2_ps[:, js[0] : js[0] + nj],
                    -A_OFF / B_SCALE,
                    None,
                    mybir.AluOpType.mult,
                )

            def evict_stage(g):
                js = GSPEC[g]
                nj = len(js)
                y_t[g] = ypool.tile([P, nj, IK], F16, name="ysb", tag="ysb")
                for jrel, j in enumerate(js):
                    nc.vector.tensor_scalar(
                        y_t[g][:, jrel, :],
                        p_t[j][:],
                        sa_t[g][:, jrel : jrel + 1],
                        -B_SCALE,
                        mybir.AluOpType.add,
                        mybir.AluOpType.mult,
                    )

            def store_stage(g):
                js = GSPEC[g]
                nc.sync.dma_start(
                    out=y_d[:, js[0] : js[0] + len(js)], in_=y_t[g][:]
                )

            # --- prologue ---
            # tiny Exp on garbage data: forces the ACT table load to run
            # before real work needs the engine
            warm = singles.tile([P, 1], F32)
            nc.scalar.activation(
                warm[:], warm[:], mybir.ActivationFunctionType.Exp,
                bias=0.0, scale=0.0,
            )
            ones_sb = singles.tile([P, 1], F16)
            nc.vector.memset(ones_sb[:], 1.0)
            s2_ps = sps.tile([P, IN_BLOCKS], F32)
            # PE warm-up: back-to-back dummy matmuls raise the HAM clock
            # gate to 2.4 GHz before the real matmuls arrive
            warm_l = singles.tile([P, 16], F16)
            nc.vector.memset(warm_l[:], 0.5)
            warm_r = singles.tile([P, IK], F16)
            nc.vector.memset(warm_r[:], 0.5)
            wm_ps = sps.tile([16, IK], F32)
            for _ in range(8):
                nc.tensor.matmul(
                    wm_ps[:], warm_l[:], warm_r[:], start=True, stop=True
                )

            # all input DMAs issued upfront: weights j-ordered on the sync
            # ring, x in two chunks on the gpsimd ring
            xt_sb = singles.tile([P, IN_BLOCKS, 2, BATCH], F16)
            nc.gpsimd.dma_start(out=xt_sb[:, 0:8], in_=xt_d[:, 0:8])
            nc.gpsimd.dma_start(out=xt_sb[:, 8:16], in_=xt_d[:, 8:16])
            for g in range(NG):
                dma_w(g)

            # --- pipelined main loop ---
            gg_stage(0)
            for g in range(NG):
                if g + 1 < NG:
                    gg_stage(g + 1)
                n_stage(g)
                exp_stage(g)
                mm_stage(g)
                sa_stage(g)
                if g >= 1:
                    evict_stage(g - 1)
                    store_stage(g - 1)
            evict_stage(NG - 1)
            store_stage(NG - 1)

    nc.compile()
    return nc


def _get_nc():
    if "nc" not in _CACHE:
        _CACHE["nc"] = _build()
    return _CACHE["nc"]


def kernel(x: np.ndarray, cores: np.ndarray, _trace=False, _trace_kwargs=None):
    x = np.asarray(x, dtype=np.float32)
    cores = np.asarray(cores, dtype=np.float32)

    xt = np.ascontiguousarray(
        x.T.reshape(IN_BLOCKS, 2, 128, BATCH)
        .transpose(2, 0, 1, 3)
        .astype(np.float16)
    )
    wt_full = (
        cores.reshape(OUT_BLOCKS, IN_BLOCKS, NB, 2, 128)  # i, j, k, h, p
        .transpose(4, 1, 3, 0, 2)  # p, j, h, i, k
        .astype(np.float16)
    )

    in_maps = []
    for c in range(N_CORES):
        wc = np.ascontiguousarray(
            wt_full[:, :, :, c * I_PER_CORE : (c + 1) * I_PER_CORE, :]
        ).reshape(128, IN_BLOCKS, 2, IK)
        in_maps.append({"xt": xt, "wt": wc})

    nc = _get_nc()
    kw = {}
    if _trace:
        kw = dict(trace=True, **(_trace_kwargs or {}))
    out = run_bass_kernel_spmd(nc, in_maps, core_ids=list(range(N_CORES)), **kw)
    if _trace:
        _CACHE["last_result"] = out
    y = np.concatenate(
        [
            r["y"]  # (b, j, (i,k))
            .astype(np.float32)
            .reshape(BATCH, IN_BLOCKS, I_PER_CORE, NB)
            .transpose(0, 2, 1, 3)
            for r in out.results
        ],
        axis=1,
    )
    return y
